# revision 22
# baseline (speedup 1.0000x reference)
"""Trainium2 Bass kernel for nn_CaptchaRecognizer (norse-style SNN).

Strategy (pure data-parallel over batch, 8 NeuronCores, 16 images each):

The encoder resets to exactly 0 on spike, so the encoder+LIF0 cascade is a
piecewise-constant function of x alone: only 4 fp32-exact breakpoints B_n
matter, and the LIF0 spike train is EXACTLY LINEAR in the 4 nested threshold
masks u_n = (x >= B_n):   z0[t] = sum_n D[n,t] * u_n   (D host-precomputed).

Hence the layer-0 LI membrane is   V0[t,b,o] = sum_n H[n,t] * Y_n[b,o]   with
Y_n = u_n @ w0^T and H = LI-filtered D.

  fast path: the reset-free LIF1 membrane is a triple first-order filter of
            V0 with kernel l1-norm <= 50. A cheap certified bound
            50 * max_b sum_n (max_t|H_n|) max_o |Y_n[b,o]| < 95 (< threshold
            100) => layer 1 never spikes => layers 2..5 exactly zero =>
            output the zero logit tile. Only the ~3% active w0 rows are
            gathered (one indirect DMA off a per-partition compaction), the
            4-mask matmul runs on a HAM-warmed PE, and the cross-partition
            reduction is a single PE transpose; the If branch target is
            prefetched via a branch hint so the skip over the dense body is
            cheap.
  slow path: runtime If; exact dense recomputation (the original per-layer
            pipeline: dense fp8 DoubleRow w0 matmul, LIF via
            scalar_tensor_tensor steps, LI via tensor_tensor_scan linear
            recurrences, bf16 matmuls for w1..w5).
  output:   max over t of V5/10, log_softmax on host (tiny [128,10]).

Internal dtypes: fp8 masks/w0 (x64), bf16 states/weights/compaction,
fp32 PSUM/x-gather.
"""

import os
import sys
import numpy as np
import ml_dtypes

import concourse.bass as bass
import concourse.tile as tile
from concourse import bacc, mybir
from concourse.bass_utils import run_bass_kernel_spmd

AL = mybir.AluOpType
F32 = mybir.dt.float32
BF16 = mybir.dt.bfloat16
FP8 = mybir.dt.float8e4
I32 = mybir.dt.int32
FP8_NP = mybir.dt.np(mybir.dt.float8e4)
BF16_NP = ml_dtypes.bfloat16
W0_SCALE = 64.0

N_CORES = 8
B_CORE = 16
T = 32
NMASK = 4

LAYER_SIZES = [(2000, 12000), (1500, 2000), (1000, 1500), (500, 1000), (100, 500), (10, 100)]
IN_PAD = [12032, 2048, 1536, 1024, 512, 128]
OUT_PAD = [2048, 1536, 1024, 512, 128, 16]
IC = [94, 16, 12, 8, 4, 1]      # input chunks of 128 (contraction)
MC = [16, 12, 8, 4, 1, 1]       # output chunks (M tiles)
M_SIZE = [128, 128, 128, 128, 128, 16]
KP0 = 47                         # layer-0 DoubleRow k-pairs
KSLOT = 10                       # gather slots per partition (max actives)
NFROW = 12160                    # padded feature rows for the gather tables
NWARM = 28                       # PE HAM warm-up dummy matmuls
THR_BF = 2.90625                 # bf16-safe activity threshold (<= fp32 B3)

LAST_EXEC_TIME_NS = None

DT_DECAY_V = np.float32(0.1)   # DT*TAU_MEM_INV
V_TH = np.float32(1.0)


def _enc_first_spike_step(x_scalar):
    """fp32 encoder sim (exactly mirrors reference arithmetic); first spike step or None."""
    f32 = np.float32
    v = f32(0.0)
    x = f32(x_scalar)
    for t in range(T):
        v = f32(v + f32(DT_DECAY_V * f32(-v + x)))
        if f32(v - V_TH) > 0:
            return t
    return None


def _stage0_tables():
    """Host-precomputed structure of the encoder+LIF0 cascade.

    The encoder resets to exactly 0 on spike, so its spike train is periodic
    with period p(x) = 1 + first_spike_step(x); LIF0's response to a period-p
    train is a fixed pattern G[t, p].  The map x -> LIF0-spike-train is
    piecewise constant in x; we compress it to the breakpoints where the
    pattern actually changes and pack patterns as integer codes.
    Returns (breaks [(B_n, delta_n)...], bit_ts [t for each bit, ascending]).
    """
    f32 = np.float32
    # G[t, c]: c = 0 -> silent input; c = p -> period p
    G = np.zeros((T, 34), np.int64)
    for c in range(1, 33):
        v = f32(0.0)
        i = f32(0.0)
        for t in range(T):
            inp = f32(1.0) if (t + 1) % c == 0 else f32(0.0)
            v_dec = f32(v + f32(DT_DECAY_V * f32(-v + i)))
            i_dec = f32(i * f32(0.8))
            z = 1 if f32(v_dec - V_TH) > 0 else 0
            v = f32(0.0) if z else v_dec
            i = f32(i_dec + inp)
            G[t, c] = z
    bit_ts = [t for t in range(T) if G[t].any()]
    code = {c: sum(int(G[ts, c]) << j for j, ts in enumerate(bit_ts)) for c in range(34)}
    code[33] = 0  # period > 32 == silent
    used = [n for n in range(1, 33) if code[n] != code[n + 1]]

    # fp32-exact breakpoints: B_n = min x with first_spike_step <= n-1
    breaks = []
    for n in used:
        lo = np.float32(1.0).view(np.int32)
        hi = np.float32(20.0).view(np.int32)
        while int(hi) - int(lo) > 1:
            mid = np.int32((int(lo) + int(hi)) // 2)
            s = _enc_first_spike_step(mid.view(np.float32))
            if s is not None and s <= n - 1:
                hi = mid
            else:
                lo = mid
        breaks.append((float(np.int32(hi).view(np.float32)), float(code[n] - code[n + 1])))
    return breaks, bit_ts


def _mask_tables():
    """Per-breakpoint spike-train deltas D [4, T] and LI-filtered H [4, T].

    z0[t] (LIF0 spikes) = sum_n (x >= B_n) * D[n, t]  exactly (nested masks).
    V0[t] (scaled LI0 membrane, V = 10*v) = sum_n H[n, t] * Y_n, with
    H the (i' = 0.8 i' + D; V = 0.9 V + i') double filter of D.
    """
    breaks, bit_ts = _stage0_tables()
    assert len(breaks) == NMASK
    deltas = [d for (_, d) in breaks]
    Bs = [b for (b, _) in breaks]
    # Bs descending: passing B_n implies passing all later (smaller) breakpoints.
    csum = np.cumsum(deltas[::-1])[::-1]  # code when masks n..3 are on

    def bits(c):
        c = int(round(c))
        return np.array([(c >> j) & 1 for j in range(len(bit_ts))], np.float64)

    pats = [bits(c) for c in csum] + [np.zeros(len(bit_ts))]
    D = np.zeros((NMASK, T))
    for n in range(NMASK):
        dv = pats[n] - pats[n + 1]
        for j, t in enumerate(bit_ts):
            D[n, t] = dv[j]
    H = np.zeros((NMASK, T))
    for n in range(NMASK):
        ip = 0.0
        V = 0.0
        for t in range(T):
            ip = 0.8 * ip + D[n, t]
            V = 0.9 * V + ip
            H[n, t] = V
    return Bs, D, H


def _install_ntff_hook():
    import types
    if "antenv.axon_hooks" in sys.modules:
        return
    try:
        mod = types.ModuleType("antenv.axon_hooks")
        mod._hook = None
        mod.set_axon_ntff_profile_hook = lambda h: setattr(mod, "_hook", h)
        mod.get_axon_ntff_profile_hook = lambda: mod._hook
        sys.modules["antenv.axon_hooks"] = mod
        from trn_agent_boot.trn_boot import _ntff_profile_via_ctypes
        mod._hook = _ntff_profile_via_ctypes("/opt/axon/libaxon_pjrt.so")
    except Exception:
        pass


ALL_ENGINES = [
    mybir.EngineType.SP,
    mybir.EngineType.Activation,
    mybir.EngineType.DVE,
    mybir.EngineType.PE,
    mybir.EngineType.Pool,
]


def build_body(tc, ctx, nc, xsb_ap, xs_ap, w_aps, h_ap, out_ap, wx_ap,
               bc_ap, fc_ap, id_ap, dbg_ap=None):
    from contextlib import ExitStack

    Bs, _D, _H = _mask_tables()
    A = np.abs(_H).max(1)  # per-channel max_t |H|

    const = ctx.enter_context(tc.tile_pool(name="const", bufs=1))
    psum = ctx.enter_context(tc.tile_pool(name="psum", bufs=8, space="PSUM"))
    ijpool = ctx.enter_context(tc.tile_pool(name="ij", bufs=2))
    spool = ctx.enter_context(tc.tile_pool(name="spikes", bufs=2))

    # prefetch hint for the fast-path branch (target = out-of-line THEN block
    # right before the merge; Else dense body is the fallthrough)
    if os.environ.get("KHINT", "1") == "1":
        tc.mark_branch_hint_location("fastskip", hint="LikelyTaken", engines=ALL_ENGINES)

    # ---- early, dependency-free setup -------------------------------------
    # zero output (fast path's result); sync engine
    zero_out = const.tile([M_SIZE[5], B_CORE], F32)
    xsb = const.tile([128, IC[0], B_CORE], BF16)    # bf16 x for activity detection
    nc.scalar.dma_start(xsb[:], xsb_ap)
    nc.vector.memset(zero_out[:], 0.0)
    nc.sync.dma_start(out_ap, zero_out[:])

    # host-built constants: cvb|kbf (bf16 blob), piof|aRow (f32 blob), ident
    bcon = const.tile([128, IC[0] + KSLOT], BF16)
    nc.sync.dma_start(bcon[:], bc_ap)
    cvb = bcon[:, 0:IC[0]]                          # c+1 (<=94, bf16-exact)
    kbf = bcon[:, IC[0]:IC[0] + KSLOT]              # slot index k
    fcon = const.tile([128, KSLOT + B_CORE * NMASK], F32)
    nc.sync.dma_start(fcon[:], fc_ap)
    piof = fcon[:, 0:KSLOT]                         # p - 128 (folds the ck-1)
    aRow = fcon[0:1, KSLOT:KSLOT + B_CORE * NMASK].rearrange(
        "p (b i) -> p b i", i=NMASK)                # 50*A_i/W0_SCALE per (b,i)
    ident = const.tile([128, 128], BF16)
    nc.sync.dma_start(ident[:], id_ap)

    ones = const.tile([128, IC[0]], BF16)
    nc.vector.memset(ones[:], 1.0)
    dmt = const.tile([128, 512], FP8)
    nc.vector.memset(dmt[:], 0.0625)
    red128 = const.tile([128, 1], BF16)
    nc.vector.memset(red128[:], 0.0)
    # combined gather landing tile: [w0 row fp8 x64 (2048B) | x f32[16] (64B)]
    wgx = const.tile([128, KSLOT, 2112], FP8)

    # ---- PE HAM warm-up: keep the array busy until the real matmuls -------
    dps = psum.tile([B_CORE * NMASK, 512], F32, name="ps0", bufs=1)
    for _ in range(NWARM):
        nc.tensor.matmul(dps[:], dmt[:, 0:64], dmt[:, :], start=True, stop=True)

    # ---- active-chunk compaction (vector, bf16) ---------------------------
    # A (p, c) chunk is active iff any of its 16 images crosses the lowest
    # breakpoint (threshold lowered to be bf16-round-safe: false positives
    # only, which gather all-zero-mask rows and contribute nothing).
    xm = const.tile([128, IC[0]], BF16)
    nc.vector.tensor_reduce(xm[:], xsb[:], mybir.AxisListType.X, AL.max)
    act = const.tile([128, IC[0]], BF16)
    nc.vector.tensor_scalar(act[:], xm[:], THR_BF, None, AL.is_ge)
    incl = const.tile([128, IC[0]], BF16)
    nc.vector.tensor_tensor_scan(incl[:], ones[:], act[:], 0.0, AL.mult, AL.add)
    acv = const.tile([128, IC[0]], BF16)
    nc.vector.tensor_tensor(acv[:], act[:], cvb[:], AL.mult)

    # ck[p, k] = (chunk index + 1) of the k-th active chunk of p, or 0 (pad);
    # (incl == k+1) holds from the k-th active onward until the next active,
    # and acv isolates the active position. Two halves so the slot-0..4
    # gathers launch while slots 5..9 are still being extracted.
    KH = KSLOT // 2
    incl_b = incl[:].unsqueeze(1).broadcast_to([128, KH, IC[0]])
    acv_b = acv[:].unsqueeze(1).broadcast_to([128, KH, IC[0]])
    fois = []
    for h in range(2):
        eqa = const.tile([128, KH, IC[0]], BF16, tag=f"eqa{h}")
        kbf_b = kbf[:, h * KH:(h + 1) * KH].unsqueeze(2).broadcast_to([128, KH, IC[0]])
        nc.vector.tensor_tensor(eqa[:], incl_b, kbf_b, AL.is_equal)
        nc.vector.tensor_tensor(eqa[:], eqa[:], acv_b, AL.mult)
        ck = const.tile([128, KH], BF16, tag=f"ck{h}")
        with nc.allow_low_precision(reason="single nonzero term <= 95, bf16-exact"):
            nc.vector.tensor_reduce(ck[:], eqa[:], mybir.AxisListType.X, AL.add)
        # pads -> chunk 94 (zero rows of the tables); row = (ck-1)*128 + p
        tmpk = const.tile([128, KH], F32, tag=f"tmpk{h}")
        nc.vector.tensor_scalar(tmpk[:], ck[:], 0.0, 95.0, AL.is_equal, AL.mult)
        ckf = const.tile([128, KH], F32, tag=f"ckf{h}")
        nc.vector.tensor_tensor(ckf[:], ck[:], tmpk[:], AL.add)
        fof = const.tile([128, KH], F32, tag=f"fof{h}")
        nc.vector.scalar_tensor_tensor(
            fof[:], ckf[:], 128.0, piof[:, h * KH:(h + 1) * KH], AL.mult, AL.add)
        foi = const.tile([128, KH], I32, tag=f"foi{h}")
        nc.vector.tensor_copy(foi[:], fof[:])
        fois.append(foi)
        # ---- per-slot gathers for this half (combined w0|x rows).
        # NOTE: multi-column offset APs ([128,K]) crash the exec unit on this
        # toolchain; only one offset per partition per instruction works. ----
        for k in range(KH):
            nc.gpsimd.indirect_dma_start(
                out=wgx[:, h * KH + k, :], out_offset=None, in_=wx_ap,
                in_offset=bass.IndirectOffsetOnAxis(ap=foi[:, k:k + 1], axis=0),
            )

    # ---- masks + 4-channel matmul (t-outer: pipelined behind the gathers) -
    uc = const.tile([128, KSLOT // 2, 2, B_CORE, NMASK], FP8)
    xgv = wgx[:, :, 2048:2112].bitcast(F32).rearrange("p (t j) b -> p t j b", j=2)
    wgv = wgx[:, :, 0:2048].rearrange("p (t j) o -> p t j o", j=2)
    rm4 = const.tile([B_CORE * NMASK, 4], F32)
    absY = const.tile([B_CORE * NMASK, 4, 512], BF16)
    ps = [psum.tile([B_CORE * NMASK, 512], F32, name=f"ps{og}", bufs=1) for og in range(4)]
    for t in range(KSLOT // 2):
        for i, bn in enumerate(Bs):
            nc.vector.tensor_scalar(uc[:, t, :, :, i], xgv[:, t], float(bn), None, AL.is_ge)
        for og in range(4):
            nc.tensor.matmul(
                ps[og][:],
                uc[:, t, :, :, :],
                wgv[:, t, :, og * 512:(og + 1) * 512],
                start=(t == 0),
                stop=(t == KSLOT // 2 - 1),
                perf_mode=mybir.MatmulPerfMode.DoubleRow,
            )
    for og in range(4):
        nc.scalar.activation(
            absY[:, og, :], ps[og][:], mybir.ActivationFunctionType.Abs, scale=1.0
        )
        nc.vector.tensor_reduce(
            rm4[:, og:og + 1], absY[:, og, :], mybir.AxisListType.X, AL.max
        )

    # ---- certified bound via one PE transpose -----------------------------
    # loose-but-valid: 50 * max_b sum_i A_i max_o |Y_i[b,o]|  (>= exact form)
    with nc.allow_low_precision(reason="max-reduce for a bound with 45% margin"):
        nc.vector.tensor_reduce(red128[0:B_CORE * NMASK, 0:1], rm4[:], mybir.AxisListType.X, AL.max)
    ovf = const.tile([128, 1], BF16)   # slot overflow forces the dense path
    nc.vector.tensor_scalar(ovf[:], incl[:, IC[0] - 1:], float(KSLOT), 8192.0, AL.is_gt, AL.mult)
    red_aug = const.tile([128, 1], BF16)
    nc.vector.tensor_tensor(red_aug[:], red128[:], ovf[:], AL.max)
    psT = psum.tile([B_CORE * NMASK, 512], F32, name="ps1", bufs=1)
    pst = psT[0:1, 0:128]
    nc.tensor.matmul(pst, red_aug[:], ident[:], start=True, stop=True)
    wtmp = const.tile([1, B_CORE, NMASK], F32)
    nc.vector.tensor_tensor(
        wtmp[:], psT[0:1, 0:B_CORE * NMASK].rearrange("p (b i) -> p b i", i=NMASK),
        aRow, AL.mult,
    )
    sb = const.tile([1, B_CORE], F32)
    nc.vector.tensor_reduce(sb[:], wtmp[:], mybir.AxisListType.X, AL.add)
    sm = const.tile([1, 1], F32)
    nc.vector.tensor_reduce(sm[:], sb[:], mybir.AxisListType.X, AL.max)
    so = const.tile([1, 1], F32)
    nc.vector.tensor_reduce(so[:], psT[0:1, 64:128], mybir.AxisListType.X, AL.max)
    svalf = const.tile([1, 1], F32)
    nc.vector.tensor_tensor(svalf[:], sm[:], so[:], AL.max)
    svi = const.tile([1, 1], I32)
    nc.vector.tensor_copy(svi[:], svalf[:])
    if dbg_ap is not None:
        nc.sync.dma_start(dbg_ap, svalf[:])
    if os.environ.get("KDBG", "0") == "1":
        foid = nc.dram_tensor("foid", [128, KSLOT], I32, kind="ExternalOutput")
        rm4d = nc.dram_tensor("rm4d", [B_CORE * NMASK, 4], F32, kind="ExternalOutput")
        pstd = nc.dram_tensor("pstd", [1, 128], F32, kind="ExternalOutput")
        nc.sync.dma_start(foid.ap(), foi[:])
        nc.sync.dma_start(rm4d.ap(), rm4[:])
        sbp = const.tile([1, 128], F32)
        nc.vector.tensor_copy(sbp[:], psT[0:1, 0:128])
        nc.sync.dma_start(pstd.ap(), sbp[:])
    _, (sval,) = nc.values_load_multi_w_load_instructions(
        svi[0:1, 0:1], skip_runtime_bounds_check=True
    )

    # ---- slow-path helpers (baseline per-layer dense pipeline) ------------
    mask08 = const.tile([128, 512], BF16)
    mask09 = const.tile([128, 512], BF16)
    xr_sb = const.tile([128, 96, B_CORE], F32)
    hc = const.tile([NMASK, T], BF16)
    Ysb = const.tile([B_CORE * NMASK, 4, 512], BF16)

    def emit_masks():
        # decay masks with 0.0 at t=0 of each batch segment (scan segmentation)
        nc.vector.memset(mask08[:], 0.8)
        nc.vector.memset(mask08[:].rearrange("p (b t) -> p b t", b=B_CORE)[:, :, 0:1], 0.0)
        nc.vector.memset(mask09[:], 0.9)
        nc.vector.memset(mask09[:].rearrange("p (b t) -> p b t", b=B_CORE)[:, :, 0:1], 0.0)

    def emit_dense_Y():
        # exact dense recomputation of Y, Else only
        with ExitStack() as phd:
            pd = phd.enter_context(tc.tile_pool(name="dense0", bufs=1))
            uf = pd.tile([128, KP0, 2, B_CORE, NMASK], FP8, tag="uf")
            xrv = xr_sb[:, :94, :].rearrange("p (a j) b -> p a j b", j=2)
            for i, bn in enumerate(Bs):
                nc.vector.tensor_scalar(uf[:, :, :, :, i], xrv, float(bn), None, AL.is_ge)
            w0pool = phd.enter_context(tc.tile_pool(name="w0s", bufs=2))
            W0_GROUPS = [(0, 2), (2, 10), (10, 18), (18, 26), (26, 34), (34, 42), (42, 47)]
            psd = [psum.tile([B_CORE * NMASK, 512], F32, name=f"ps{og}", bufs=1) for og in range(4)]
            for g0, g1 in W0_GROUPS:
                wt = w0pool.tile([128, 8, 2, 2048], FP8, name="wt")
                nc.sync.dma_start(
                    wt[:, :g1 - g0, :, :],
                    w_aps[0][g0:g1].rearrange("g p j o -> p g j o"),
                )
                for kp in range(g0, g1):
                    for og in range(4):
                        nc.tensor.matmul(
                            psd[og][:],
                            uf[:, kp, :, :, :],
                            wt[:, kp - g0, :, og * 512:(og + 1) * 512],
                            start=(kp == 0),
                            stop=(kp == KP0 - 1),
                            perf_mode=mybir.MatmulPerfMode.DoubleRow,
                        )
            for og in range(4):
                nc.vector.tensor_scalar(
                    Ysb[:, og, :], psd[og][:], 1.0 / W0_SCALE, None, AL.mult
                )

    spikes = None  # current layer's input spike tensor, [128, IC[k], 16, 32] bf16

    def lif_phase(k, V, pk):
        nonlocal spikes
        C = MC[k]
        Vv = V[:].rearrange("p m (b t) -> p m b t", t=T)
        S = spool.tile([128, C, B_CORE, T], BF16, tag="S")
        P = pk.tile([128, C, B_CORE], BF16, tag="P")
        Q = pk.tile([128, C, B_CORE], BF16, tag="Q")
        nc.vector.memset(P[:], 0.0)
        nc.vector.memset(Q[:], 0.0)
        for t in range(T):
            nc.vector.scalar_tensor_tensor(P[:], P[:], 0.9, Q[:], AL.mult, AL.add)
            nc.vector.tensor_scalar(S[:, :, :, t], P[:], 100.0, None, AL.is_gt)
            nc.vector.scalar_tensor_tensor(P[:], P[:], 100.0, P[:], AL.is_le, AL.mult)
            nc.vector.scalar_tensor_tensor(Q[:], Q[:], 0.8, Vv[:, :, :, t], AL.mult, AL.add)
        spikes = S

    def layer_phase(k):
        nonlocal spikes
        M = M_SIZE[k]
        with ExitStack() as ph:
            pk = ph.enter_context(tc.tile_pool(name=f"phase{k + 1}", bufs=1))
            if k == 5:
                V = pk.tile([M, 512], F32, tag="V5")
            else:
                V = pk.tile([128, MC[k], 512], BF16, tag=f"V{k}")

            wk_sb = pk.tile([128, IC[k], OUT_PAD[k]], BF16, tag=f"w{k}")
            nc.sync.dma_start(wk_sb[:], w_aps[k])

            for m in range(MC[k]):
                psl = psum.tile([128, 512], F32, bufs=2)
                for kc in range(IC[k]):
                    nc.tensor.matmul(
                        psl[:M, :],
                        wk_sb[:, kc, m * 128:m * 128 + M],
                        spikes[:, kc, :, :],
                        start=(kc == 0),
                        stop=(kc == IC[k] - 1),
                    )
                j_src = psl[:M, :]
                ij = ijpool.tile([128, 512], BF16)
                nc.vector.tensor_tensor_scan(ij[:M, :], mask08[:M, :], j_src, 0.0, AL.mult, AL.add)
                if k == 5:
                    nc.vector.tensor_tensor_scan(V[:, :], mask09[:M, :], ij[:M, :], 0.0, AL.mult, AL.add)
                else:
                    nc.vector.tensor_tensor_scan(V[:, m, :], mask09[:, :], ij[:, :], 0.0, AL.mult, AL.add)

            if k == 1 and os.environ.get("KDBG2", "0") == "1":
                v1d = nc.dram_tensor("v1d", [128, MC[1], 512], BF16, kind="ExternalOutput")
                nc.sync.dma_start(v1d.ap(), V[:])
            if k == 5:
                rmax = pk.tile([M, B_CORE], F32)
                nc.vector.tensor_reduce(
                    rmax[:], V[:].rearrange("p (b t) -> p b t", b=B_CORE),
                    mybir.AxisListType.X, AL.max,
                )
                nc.sync.dma_start(out_ap, rmax[:])
            else:
                lif_phase(k, V, pk)

    if os.environ.get("KHINT", "1") == "1":
        ifctx = tc.If(sval < 95, preferred_fallthrough_block=False, label="fastskip")
    else:
        ifctx = tc.If(sval < 95)
    with ifctx as cmp:
        pass
    with cmp.Else():
        # dense exact fallback (never taken when the certificate holds)
        nc.vector.memset(xr_sb[:, 94:96, :], 0.0)
        nc.sync.dma_start(xr_sb[:, :94, :].rearrange("p (a j) b -> p a j b", j=2), xs_ap)
        nc.sync.dma_start(hc[:], h_ap)
        emit_masks()
        emit_dense_Y()
        with ExitStack() as phl:
            pl = phl.enter_context(tc.tile_pool(name="lif1", bufs=1))
            # transpose Y to partitions = i for PE expansion against H
            Yt = pl.tile([NMASK, B_CORE, 4, 512], BF16, tag="Yt")
            for b in range(B_CORE):
                nc.sync.dma_start(
                    Yt[:, b, :, :], Ysb[b * NMASK:(b + 1) * NMASK, :, :]
                )
            V0 = pl.tile([128, MC[0], 512], BF16, tag="V0")
            for m in range(MC[0]):
                psv = psum.tile([128, 512], F32, bufs=1)
                for b in range(B_CORE):
                    nc.tensor.matmul(
                        psv[:, b * T:(b + 1) * T],
                        Yt[:, b, m // 4, (m % 4) * 128:(m % 4) * 128 + 128],
                        hc[:],
                        start=True, stop=True,
                    )
                nc.scalar.activation(
                    V0[:, m, :], psv[:], mybir.ActivationFunctionType.Copy, scale=1.0
                )
            lif_phase(0, V0, pl)
        for k in range(1, 6):
            layer_phase(k)


def build_nc():
    from contextlib import ExitStack

    nc = bacc.Bacc("TRN2", debug=False, num_devices=N_CORES)
    xsb = nc.dram_tensor("xsb", [128, IC[0], B_CORE], BF16, kind="ExternalInput")
    xs = nc.dram_tensor("xs", [128, KP0, 2, B_CORE], F32, kind="ExternalInput")
    w_t = [nc.dram_tensor("w0t", [KP0, 128, 2, 2048], FP8, kind="ExternalInput")]
    for k in range(1, 6):
        w_t.append(
            nc.dram_tensor(f"w{k}t", [128, IC[k], OUT_PAD[k]], BF16, kind="ExternalInput")
        )
    hconst = nc.dram_tensor("hconst", [NMASK, T], BF16, kind="ExternalInput")
    bconst = nc.dram_tensor("bconst", [128, IC[0] + KSLOT], BF16, kind="ExternalInput")
    fconst = nc.dram_tensor("fconst", [128, KSLOT + B_CORE * NMASK], F32, kind="ExternalInput")
    identc = nc.dram_tensor("identc", [128, 128], BF16, kind="ExternalInput")
    wxgath = nc.dram_tensor("wxgath", [NFROW, 2112], FP8, kind="ExternalInput")
    out = nc.dram_tensor("out", [M_SIZE[5], B_CORE], F32, kind="ExternalOutput")
    dbg = nc.dram_tensor("dbg", [1, 1], F32, kind="ExternalOutput")

    with tile.TileContext(nc) as tc, ExitStack() as ctx:
        build_body(tc, ctx, nc, xsb.ap(), xs.ap(), [w.ap() for w in w_t],
                   hconst.ap(), out.ap(), wxgath.ap(),
                   bconst.ap(), fconst.ap(), identc.ap(),
                   dbg_ap=dbg.ap())
    nc.compile()
    return nc


def prep_inputs(images, ws):
    """Host-side marshalling: pad/transpose/cast weights, rearrange images."""
    x = np.asarray(images).reshape(128, -1).astype(np.float32)  # [B, 12000]
    xs = np.zeros((128, 12032), np.float32)
    xs[:, :12000] = x
    # f32 for the dense fallback: [p, kp, j, b] with feature f = (2*kp+j)*128+p
    xs_r = xs.reshape(128, 47, 2, 128).transpose(3, 1, 2, 0)  # [128p, 47, 2, 128b]
    xs_cores = [
        np.ascontiguousarray(xs_r[:, :, :, c * B_CORE:(c + 1) * B_CORE])
        for c in range(N_CORES)
    ]
    # bf16 for activity detection: [p, a, b] with a = 2*kp + j
    xsb_r = xs.astype(BF16_NP).reshape(128, 94, 128).transpose(2, 1, 0)
    xsb_cores = [
        np.ascontiguousarray(xsb_r[:, :, c * B_CORE:(c + 1) * B_CORE])
        for c in range(N_CORES)
    ]

    w_prepped = []
    wT0 = np.zeros((12032, 2048), np.float32)
    wT0[:12000, :2000] = np.asarray(ws[0]).T * np.float32(W0_SCALE)
    # [47 kp, 128 p, 2 j, 2048 o]: feature f = (2*kp + j)*128 + p
    w0p = wT0.reshape(47, 2, 128, 2048).transpose(0, 2, 1, 3)
    w_prepped.append(np.ascontiguousarray(w0p.astype(FP8_NP)))
    for k in range(1, 6):
        out_f, in_f = LAYER_SIZES[k]
        wTk = np.zeros((IN_PAD[k], OUT_PAD[k]), np.float32)
        wTk[:in_f, :out_f] = np.asarray(ws[k]).T
        wkp = wTk.reshape(IC[k], 128, OUT_PAD[k]).transpose(1, 0, 2)  # [128p, IC, OUT]
        w_prepped.append(np.ascontiguousarray(wkp.astype(BF16_NP)))

    wx_base = np.zeros((NFROW, 2112), np.uint8)
    wx_base[:12032, :2048] = wT0.astype(FP8_NP).view(np.uint8)
    wx_cores = []
    for c in range(N_CORES):
        t = wx_base.copy()
        t[:12032, 2048:] = np.ascontiguousarray(
            xs[c * B_CORE:(c + 1) * B_CORE].T).view(np.uint8).reshape(12032, 64)
        wx_cores.append(t.view(FP8_NP))
    _Bs, _D, H = _mask_tables()
    hmat = np.ascontiguousarray(H.astype(BF16_NP))  # [4, 32]
    A = np.abs(H).max(1)
    bcon = np.zeros((128, IC[0] + KSLOT), np.float32)
    bcon[:, :IC[0]] = np.arange(1, IC[0] + 1, dtype=np.float32)[None, :]
    bcon[:, IC[0]:] = np.arange(1, KSLOT + 1, dtype=np.float32)[None, :]
    bcon = np.ascontiguousarray(bcon.astype(BF16_NP))
    fcon = np.zeros((128, KSLOT + B_CORE * NMASK), np.float32)
    fcon[:, :KSLOT] = (np.arange(128, dtype=np.float32) - 128.0)[:, None]
    arow = np.tile((50.0 * A / W0_SCALE).astype(np.float32), B_CORE)  # [(b,i)] i-minor
    fcon[0, KSLOT:] = arow
    fcon = np.ascontiguousarray(fcon)
    identm = np.ascontiguousarray(np.eye(128, dtype=np.float32).astype(BF16_NP))
    return (xs_cores, xsb_cores, w_prepped, hmat, wx_cores,
            bcon, fcon, identm)


_NC_CACHE = {}


def kernel(images, w0, w1, w2, w3, w4, w5):
    global LAST_EXEC_TIME_NS
    ws = [w0, w1, w2, w3, w4, w5]
    (xs_cores, xsb_cores, w_prepped, hmat, wx_cores,
     bcon, fcon, identm) = prep_inputs(images, ws)

    trace = os.environ.get("KERNEL_TRACE", "0") == "1"
    if trace:
        _install_ntff_hook()

    if "nc" not in _NC_CACHE:
        _NC_CACHE["nc"] = build_nc()
    nc = _NC_CACHE["nc"]

    in_maps = []
    for c in range(N_CORES):
        m = {"xs": xs_cores[c], "xsb": xsb_cores[c], "w0t": w_prepped[0],
             "hconst": hmat, "wxgath": wx_cores[c],
             "bconst": bcon, "fconst": fcon, "identc": identm}
        for k in range(1, 6):
            m[f"w{k}t"] = w_prepped[k]
        in_maps.append(m)

    res = run_bass_kernel_spmd(
        nc, in_maps, core_ids=list(range(N_CORES)), trace=trace
    )
    LAST_EXEC_TIME_NS = res.exec_time_ns
    _NC_CACHE["res"] = res

    # out[c] is [16 feats, 16 batch]; valid feats :10; logits = max_t(V5)/10
    logits = np.concatenate(
        [np.asarray(res.results[c]["out"])[:10, :].T for c in range(N_CORES)], axis=0
    ).astype(np.float32) / np.float32(10.0)
    mx = logits.max(axis=1, keepdims=True)
    sh = logits - mx
    out = sh - np.log(np.exp(sh).sum(axis=1, keepdims=True))
    return out.astype(np.float32)
